# revision 1
# baseline (speedup 1.0000x reference)
"""Trainium2 Bass kernel for ChannelAttention.

    k      = einsum('bcit,i->bct', signals, alpha)          # [B, C, T]
    scores = einsum('bct,ts,bds->bcd', k, Wc, k)            # [B, C, C]
    att    = softmax(scores, axis=-1)
    out    = einsum('bci,bint->bcnt', att, signals)         # [B, C, N, T]

Sharding: data-parallel over batch B=16 across 8 cores (2 batch elements per
core); Wc/alpha replicated; no collectives.

Per-core program (batches b0, b1 packed into 128 partitions as (b, c) pairs):
  Phase A: kT[t, b*64+c] accumulated on PE: lhsT = sig[b, c, i-chunk, :]
           ([i, t] natural layout), rhs = alpha chunk [i, 1]; 4-chunk PSUM
           accumulation per column.
  Phase B: kWT = (lhsT=Wc) @ kT; scores = (lhsT=kWT) @ kT (block-diag valid);
           row softmax per 64x64 batch block; PE transpose -> attT.
  Phase C: out tiles = (lhsT=attT) @ sig tiles [(b c), (n t)-chunk] streamed,
           PSUM -> SBUF -> HBM.

Built on bacc.Bacc: its compile() pass splits multi-semaphore waits into
event-semaphore instructions (TRN2 allows only 1 wait per instruction).
"""

import numpy as np
from contextlib import ExitStack

import concourse.bass as bass
import concourse.bacc as bacc
import concourse.tile as tile
import concourse.mybir as mybir
from concourse.bass_utils import run_bass_kernel_spmd
from concourse.masks import make_identity

B, C, N, T = 16, 64, 512, 128
NCORES = 8
BPC = B // NCORES          # batches per core
P = 128
NT = N * T                 # 65536 free elements per (b, c) channel
F = 2048                   # phase-C free chunk (8 KiB / partition per tile)
MM_F = 512                 # fp32 matmul moving-operand max free dim
FP32 = mybir.dt.float32
F32R = mybir.dt.float32r

_PROGRAM_CACHE = {}


def _build_program() -> bass.Bass:
    nc = bacc.Bacc(None)
    sig_h = nc.declare_dram_parameter("signals", [BPC, C, N, T], FP32, isOutput=False)
    wc_h = nc.declare_dram_parameter("Wc", [T, T], FP32, isOutput=False)
    al_h = nc.declare_dram_parameter("alpha", [N], FP32, isOutput=False)
    out_h = nc.declare_dram_parameter("out", [BPC, C, N, T], FP32, isOutput=True)

    sig = sig_h.ap()
    out = out_h.ap()
    sig_flat = sig.rearrange("b c n t -> (b c) (n t)")
    out_flat = out.rearrange("b c n t -> (b c) (n t)")

    with ExitStack() as ctx:
        tc = ctx.enter_context(tile.TileContext(nc))
        singles = ctx.enter_context(tc.tile_pool(name="singles", bufs=1))
        apool = ctx.enter_context(tc.tile_pool(name="apool", bufs=2))
        cpool = ctx.enter_context(tc.tile_pool(name="cpool", bufs=4))
        opool = ctx.enter_context(tc.tile_pool(name="opool", bufs=4))
        small = ctx.enter_context(tc.tile_pool(name="small", bufs=1))
        pk = ctx.enter_context(tc.tile_pool(name="pk", bufs=1, space="PSUM"))
        pb = ctx.enter_context(tc.tile_pool(name="pb", bufs=1, space="PSUM"))
        po = ctx.enter_context(tc.tile_pool(name="po", bufs=3, space="PSUM"))

        # --- constants
        wc_sb = singles.tile([T, T], FP32)
        nc.sync.dma_start(out=wc_sb, in_=wc_h.ap())
        alpha_sb = singles.tile([P, N // P], F32R)
        nc.sync.dma_start(
            out=alpha_sb,
            in_=al_h.ap().rearrange("(o p) -> p o", p=P).bitcast(F32R),
        )
        ident = singles.tile([P, P], FP32)
        make_identity(nc, ident)

        # --- Phase A: k rows on partition 0 via alpha-stationary f32r matmuls
        # (M=1, free=512), then kT assembly via PE transposes of [1,128] blocks.
        QC = 16               # channels per psum block: [1, QC*T] = 4 banks
        NQ = C // QC
        n_ichunks = N // P
        k_sb = small.tile([1, BPC * C * T], FP32)
        for b in range(BPC):
            for q in range(NQ):
                kr_ps = pk.tile([1, QC * T], FP32, tag="kr")
                for ic in range(n_ichunks):
                    a_sub = apool.tile([P, QC, T], F32R, tag="a")
                    nc.sync.dma_start(
                        out=a_sub,
                        in_=sig[b, q * QC:(q + 1) * QC, ic * P:(ic + 1) * P, :]
                        .rearrange("c i t -> i c t").bitcast(F32R),
                    )
                    a_flat = a_sub.rearrange("i c t -> i (c t)")
                    for j in range(QC * T // MM_F):
                        nc.tensor.matmul(
                            kr_ps[:, j * MM_F:(j + 1) * MM_F],
                            lhsT=alpha_sb[:, ic:ic + 1],
                            rhs=a_flat[:, j * MM_F:(j + 1) * MM_F],
                            start=(ic == 0),
                            stop=(ic == n_ichunks - 1),
                        )
                nc.vector.tensor_copy(
                    k_sb[:, (b * C + q * QC) * T:(b * C + (q + 1) * QC) * T],
                    kr_ps,
                )

        kt_ps = pb.tile([P, P], FP32, tag="pb")
        for col in range(P):
            nc.tensor.transpose(
                kt_ps[:, col:col + 1],
                k_sb[:, col * T:(col + 1) * T],
                ident[0:1, 0:1],
            )
        kt_sb = small.tile([P, P], FP32)
        nc.vector.tensor_copy(kt_sb, kt_ps)

        # --- Phase B: scores + softmax + transpose
        kwt_ps = pb.tile([P, P], FP32, tag="pb")
        nc.tensor.matmul(kwt_ps, lhsT=wc_sb, rhs=kt_sb, start=True, stop=True)
        kwt_sb = small.tile([P, P], FP32)
        nc.vector.tensor_copy(kwt_sb, kwt_ps)

        sc_ps = pb.tile([P, P], FP32, tag="pb")
        nc.tensor.matmul(sc_ps, lhsT=kwt_sb, rhs=kt_sb, start=True, stop=True)

        att = small.tile([P, P], FP32)
        nc.scalar.memzero(att)
        mx = small.tile([P, 1], FP32)
        nmx = small.tile([P, 1], FP32)
        ssum = small.tile([P, 1], FP32)
        rsum = small.tile([P, 1], FP32)
        for b in range(BPC):
            rows = slice(b * C, (b + 1) * C)
            cols = slice(b * C, b * C + C)
            blk = sc_ps[rows, cols]
            nc.vector.reduce_max(out=mx[rows], in_=blk, axis=mybir.AxisListType.X)
            nc.vector.tensor_scalar_mul(nmx[rows], mx[rows], -1.0)
            nc.scalar.activation(
                att[rows, cols], blk, mybir.ActivationFunctionType.Exp,
                bias=nmx[rows], scale=1.0, accum_out=ssum[rows],
            )
        nc.vector.reciprocal(rsum, ssum)
        for b in range(BPC):
            rows = slice(b * C, (b + 1) * C)
            cols = slice(b * C, b * C + C)
            nc.scalar.mul(att[rows, cols], att[rows, cols], rsum[rows])

        attt_ps = pb.tile([P, P], FP32, tag="pb")
        nc.tensor.transpose(attt_ps, att, ident)
        attt_sb = small.tile([P, P], F32R)
        nc.vector.tensor_copy(attt_sb, attt_ps)

        # --- Phase C: out = (lhsT=attT) @ sig, streamed over (n t)
        for f in range(NT // F):
            c_tile = cpool.tile([P, F], F32R, tag="c")
            nc.sync.dma_start(
                out=c_tile, in_=sig_flat[:, f * F:(f + 1) * F].bitcast(F32R)
            )
            o_tile = opool.tile([P, F], FP32, tag="o")
            for j in range(F // MM_F):
                o_ps = po.tile([P, MM_F], FP32, tag="po")
                nc.tensor.matmul(
                    o_ps, lhsT=attt_sb, rhs=c_tile[:, j * MM_F:(j + 1) * MM_F],
                    start=True, stop=True,
                )
                nc.vector.tensor_copy(o_tile[:, j * MM_F:(j + 1) * MM_F], o_ps)
            nc.scalar.dma_start(out=out_flat[:, f * F:(f + 1) * F], in_=o_tile)

    nc.compile()
    return nc


def _get_program() -> bass.Bass:
    if "nc" not in _PROGRAM_CACHE:
        _PROGRAM_CACHE["nc"] = _build_program()
    return _PROGRAM_CACHE["nc"]


def kernel(signals, Wc, alpha, **run_kwargs):
    signals = np.ascontiguousarray(np.asarray(signals, dtype=np.float32))
    Wc = np.ascontiguousarray(np.asarray(Wc, dtype=np.float32))
    alpha = np.ascontiguousarray(np.asarray(alpha, dtype=np.float32))
    assert signals.shape == (B, C, N, T)

    nc = _get_program()
    core_ids = list(range(NCORES))
    in_maps = [
        {
            "signals": signals[j * BPC:(j + 1) * BPC],
            "Wc": Wc,
            "alpha": alpha,
        }
        for j in range(NCORES)
    ]
    res = run_bass_kernel_spmd(nc, in_maps, core_ids, **run_kwargs)
    out = np.empty((B, C, N, T), dtype=np.float32)
    for j in range(NCORES):
        out[j * BPC:(j + 1) * BPC] = res.results[j]["out"]
    if run_kwargs:
        kernel.last_results = res
    return out



# revision 2
# speedup vs baseline: 1.0757x; 1.0757x over previous
"""Trainium2 Bass kernel for ChannelAttention, v2 (single-read + fp16 store).

    k      = einsum('bcit,i->bct', signals, alpha)          # [B, C, T]
    scores = einsum('bct,ts,bds->bcd', k, Wc, k)            # [B, C, C]
    att    = softmax(scores, axis=-1)
    out    = einsum('bci,bint->bcnt', att, signals)         # [B, C, N, T]

Sharding: data-parallel over batch B=16 across 8 cores (2 per core).

Per-core plan (vs the 96 MiB/core double-read baseline):
  - signals read from HBM ONCE (32 MiB fp32), streamed into a per-batch
    fp16 SBUF resident tile R[(h c)=128, (n' t)=32768], n = h*256 + n'.
  - Phase A (k) on the PE from R: per n'-block n0, a small matmul with
    stationary selector sel_n0[p=(h c), c'] = alpha[h*256+n0] * (c==c'),
    accumulating k[c',t] over 256 blocks in PSUM.  sel table (4 MiB fp16)
    is built on-device from alpha via a PSUM broadcast matmul + 256 DVE
    tensor_scalar ops (interleaved with the stream).
  - Phase B: kT via PE transpose, scores = (k Wc) k^T, row softmax,
    att -> fp16, attT packed block-diagonally into [128,128] (both
    n-halves at once).
  - Phase C: out[(h c), :] = attT2.T @ R in 512-col matmuls; PSUM ->
    fp16 staging (ACT/DVE copies) -> HBM as fp16 (host upcasts).
  HBM traffic/core: 32 MiB in + 16 MiB out = 48 MiB (~134 us floor at
  358 GB/s) vs 96 MiB before.

Emission order is chosen for per-engine FIFO cleanliness:
  b0 stream+A | b0 B | b1 stream (DMA+downcast only) | b0 C (ACT copies
  only) | b1 A matmuls | b1 B | b1 C (DVE+ACT copies).
"""

import numpy as np
from contextlib import ExitStack

import concourse.bass as bass
import concourse.bacc as bacc
import concourse.tile as tile
import concourse.mybir as mybir
from concourse.bass_utils import run_bass_kernel_spmd
from concourse.masks import make_identity

B, C, N, T = 16, 64, 512, 128
NCORES = 8
BPC = B // NCORES
P = 128
NH = N // 2                # 256 n'-values per half
NT2 = NH * T               # 32768 cols per resident tile
CK = 2048                  # stream chunk cols (16 n'-blocks)
NCHUNK = NT2 // CK         # 16 chunks per batch
BLK = CK // T              # 16 n'-blocks per chunk
FP32 = mybir.dt.float32
FP16 = mybir.dt.float16

_PROGRAM_CACHE = {}


def _build_program() -> bass.Bass:
    nc = bacc.Bacc(None)
    sig_h = nc.declare_dram_parameter("signals", [BPC, C, N, T], FP32, isOutput=False)
    wc_h = nc.declare_dram_parameter("Wc", [T, T], FP32, isOutput=False)
    al_h = nc.declare_dram_parameter("alpha", [N], FP32, isOutput=False)
    out_h = nc.declare_dram_parameter("out", [BPC, C, N, T], FP16, isOutput=True)

    # per-batch, per-half [c, (n' t)] views; n = h*256 + n'.  SBUF rows
    # (h*64 + c) are fed by two DMAs per chunk, one per half.
    sig_hc = [
        sig_h.ap()[b].rearrange("c (h np) t -> h c (np t)", h=2)
        for b in range(BPC)
    ]
    out_hc = [
        out_h.ap()[b].rearrange("c (h np) t -> h c (np t)", h=2)
        for b in range(BPC)
    ]

    with ExitStack() as ctx:
        tc = ctx.enter_context(tile.TileContext(nc))
        singles = ctx.enter_context(tc.tile_pool(name="singles", bufs=1))
        rpool = ctx.enter_context(tc.tile_pool(name="rpool", bufs=2))
        opool = ctx.enter_context(tc.tile_pool(name="opool", bufs=2))
        small = ctx.enter_context(tc.tile_pool(name="small", bufs=2))
        pa = ctx.enter_context(tc.tile_pool(name="pa", bufs=1, space="PSUM"))
        pk = ctx.enter_context(tc.tile_pool(name="pk", bufs=1, space="PSUM"))
        pb = ctx.enter_context(tc.tile_pool(name="pb", bufs=2, space="PSUM"))
        po = ctx.enter_context(tc.tile_pool(name="po", bufs=4, space="PSUM"))

        # ---- constants
        wc_sb = singles.tile([T, T], FP32)
        nc.sync.dma_start(out=wc_sb, in_=wc_h.ap())
        al_sb = singles.tile([1, N], FP32)
        nc.sync.dma_start(out=al_sb, in_=al_h.ap().rearrange("(o n) -> o n", o=1))

        id64f = singles.tile([64, 64], FP32)
        make_identity(nc, id64f)
        id64h = singles.tile([64, 64], FP16)
        make_identity(nc, id64h)

        # D[p, c'] = (p % 64 == c'), two stacked I64
        d_sb = singles.tile([P, 64], FP16)
        nc.vector.tensor_copy(d_sb[0:64, :], id64h)
        nc.vector.tensor_copy(d_sb[64:128, :], id64h)

        # a2[p, n0] = alpha[(p//64)*256 + n0] via two K=1 broadcast matmuls
        mask0 = singles.tile([1, P], FP32)
        mask1 = singles.tile([1, P], FP32)
        nc.vector.memset(mask0, 0.0)
        nc.vector.memset(mask0[0:1, 0:64], 1.0)
        nc.vector.memset(mask1, 0.0)
        nc.vector.memset(mask1[0:1, 64:128], 1.0)
        a2_ps = pa.tile([P, NH], FP32, tag="pa")
        nc.tensor.matmul(a2_ps, lhsT=mask0, rhs=al_sb[:, 0:NH], start=True, stop=False)
        nc.tensor.matmul(a2_ps, lhsT=mask1, rhs=al_sb[:, NH:N], start=False, stop=True)
        a2_sb = singles.tile([P, NH], FP32)
        nc.vector.tensor_copy(a2_sb, a2_ps)

        # selector table: sel[p, n0*64 + c'] = a2[p, n0] * D[p, c']
        sel_sb = singles.tile([P, NH * 64], FP16)

        resid = []   # per-batch resident tiles

        CKB = 8192               # big-chunk cols (32 KiB/row descriptors)
        NBCH = NT2 // CKB        # 4 chunks per batch

        def stream_batch(b, with_sel, with_mms):
            R = rpool.tile([P, NT2], FP16, tag="R", name=f"R{b}")
            resid.append(R)
            kp = pk.tile([64, T], FP32, tag="k", name=f"k{b}") if with_mms else None
            for j in range(NBCH):
                if with_sel:
                    # sel[p, n0*64+c'] = a2[p, n0] * D[p, c'], quarter at a time
                    nq = NH // NBCH
                    q0 = j * nq
                    sv = sel_sb[:, q0 * 64:(q0 + nq) * 64].rearrange(
                        "p (n c) -> p n c", n=nq
                    )
                    nc.vector.tensor_tensor(
                        sv,
                        a2_sb[:, q0:q0 + nq].unsqueeze(2).broadcast_to([P, nq, 64]),
                        d_sb.unsqueeze(1).broadcast_to([P, nq, 64]),
                        mybir.AluOpType.mult,
                    )
                # casting SWDGE DMA: HBM fp32 -> SBUF fp16, no staging
                nc.gpsimd.dma_start(
                    out=R[0:64, j * CKB:(j + 1) * CKB],
                    in_=sig_hc[b][0, :, j * CKB:(j + 1) * CKB],
                )
                nc.gpsimd.dma_start(
                    out=R[64:128, j * CKB:(j + 1) * CKB],
                    in_=sig_hc[b][1, :, j * CKB:(j + 1) * CKB],
                )
                if with_mms:
                    phase_a_chunk(b, R, kp, j)
            return R, kp

        def phase_a_chunk(b, R, kp, j):
            nblk = CKB // T
            for u in range(nblk):
                n0 = j * nblk + u
                nc.tensor.matmul(
                    kp,
                    lhsT=sel_sb[:, n0 * 64:(n0 + 1) * 64],
                    rhs=R[:, n0 * T:(n0 + 1) * T],
                    start=(n0 == 0),
                    stop=(n0 == NH - 1),
                )

        def phase_b(b, kp):
            k_sb = small.tile([64, T], FP32, tag="ksb", name=f"ksb{b}")
            nc.vector.tensor_copy(k_sb, kp)
            ktp = pb.tile([T, 64], FP32, tag="pb", name=f"ktp{b}")
            nc.tensor.transpose(ktp, k_sb, id64f)
            kt_sb = small.tile([T, 64], FP32, tag="ktsb", name=f"ktsb{b}")
            nc.vector.tensor_copy(kt_sb, ktp)
            kwtp = pb.tile([T, 64], FP32, tag="pb", name=f"kwtp{b}")
            nc.tensor.matmul(kwtp, lhsT=wc_sb, rhs=kt_sb, start=True, stop=True)
            kwt_sb = small.tile([T, 64], FP32, tag="kwtsb", name=f"kwtsb{b}")
            nc.vector.tensor_copy(kwt_sb, kwtp)
            scp = pb.tile([64, 64], FP32, tag="pb", name=f"scp{b}")
            nc.tensor.matmul(scp, lhsT=kwt_sb, rhs=kt_sb, start=True, stop=True)

            mx = small.tile([64, 1], FP32, tag="mx", name=f"mx{b}")
            nmx = small.tile([64, 1], FP32, tag="nmx", name=f"nmx{b}")
            ssum = small.tile([64, 1], FP32, tag="ssum", name=f"ssum{b}")
            rsum = small.tile([64, 1], FP32, tag="rsum", name=f"rsum{b}")
            att_f = small.tile([64, 64], FP32, tag="attf", name=f"attf{b}")
            att_h_f32 = small.tile([64, 64], FP32, tag="atth", name=f"atth{b}")
            nc.vector.reduce_max(out=mx, in_=scp, axis=mybir.AxisListType.X)
            nc.vector.tensor_scalar_mul(nmx, mx, -1.0)
            nc.scalar.activation(
                att_f, scp, mybir.ActivationFunctionType.Exp,
                bias=nmx, scale=1.0, accum_out=ssum,
            )
            nc.vector.reciprocal(rsum, ssum)
            nc.scalar.mul(att_h_f32, att_f, rsum)
            atp = pb.tile([64, 64], FP32, tag="pb", name=f"atp{b}")
            nc.tensor.transpose(atp, att_h_f32, id64f)
            at2 = small.tile([P, P], FP16, tag="at2", name=f"at2{b}")
            nc.vector.memset(at2, 0.0)
            nc.vector.tensor_copy(at2[0:64, 0:64], atp)
            nc.vector.tensor_copy(at2[64:128, 64:128], atp)
            return at2

        GRP = 4096           # out staging cols (fp16)
        QPG = GRP // 512     # 512-col matmuls per group

        def phase_c(b, R, at2, copy_engines):
            for g in range(NT2 // GRP):
                stg = opool.tile([P, GRP], FP16, tag="o", name=f"o{b}_{g}")
                for q in range(QPG):
                    ps = po.tile([P, 512], FP32, tag="po", name=f"po{b}_{g}_{q}")
                    nc.tensor.matmul(
                        ps, lhsT=at2,
                        rhs=R[:, (g * QPG + q) * 512:(g * QPG + q + 1) * 512],
                        start=True, stop=True,
                    )
                    eng = copy_engines[q % len(copy_engines)]
                    if eng == "v":
                        nc.vector.tensor_copy(stg[:, q * 512:(q + 1) * 512], ps)
                    else:
                        nc.scalar.copy(stg[:, q * 512:(q + 1) * 512], ps)
                nc.sync.dma_start(
                    out=out_hc[b][0, :, g * GRP:(g + 1) * GRP], in_=stg[0:64, :]
                )
                nc.scalar.dma_start(
                    out=out_hc[b][1, :, g * GRP:(g + 1) * GRP], in_=stg[64:128, :]
                )

        # ---- emission schedule
        R0, kp0 = stream_batch(0, with_sel=True, with_mms=True)
        at2_0 = phase_b(0, kp0)
        R1, _ = stream_batch(1, with_sel=False, with_mms=False)
        phase_c(0, R0, at2_0, copy_engines=["v", "s"])
        kp1 = pk.tile([64, T], FP32, tag="k", name="k1")
        for j in range(NBCH):
            phase_a_chunk(1, R1, kp1, j)
        at2_1 = phase_b(1, kp1)
        phase_c(1, R1, at2_1, copy_engines=["v", "s"])

    nc.compile()
    return nc


def _get_program() -> bass.Bass:
    if "nc" not in _PROGRAM_CACHE:
        _PROGRAM_CACHE["nc"] = _build_program()
    return _PROGRAM_CACHE["nc"]


def kernel(signals, Wc, alpha, **run_kwargs):
    signals = np.ascontiguousarray(np.asarray(signals, dtype=np.float32))
    Wc = np.ascontiguousarray(np.asarray(Wc, dtype=np.float32))
    alpha = np.ascontiguousarray(np.asarray(alpha, dtype=np.float32))
    assert signals.shape == (B, C, N, T)

    nc = _get_program()
    core_ids = list(range(NCORES))
    in_maps = [
        {
            "signals": signals[j * BPC:(j + 1) * BPC],
            "Wc": Wc,
            "alpha": alpha,
        }
        for j in range(NCORES)
    ]
    res = run_bass_kernel_spmd(nc, in_maps, core_ids, **run_kwargs)
    out = np.empty((B, C, N, T), dtype=np.float32)
    for j in range(NCORES):
        out[j * BPC:(j + 1) * BPC] = np.asarray(res.results[j]["out"], dtype=np.float32)
    if run_kwargs:
        kernel.last_results = res
    return out


# revision 3
# speedup vs baseline: 1.1875x; 1.1039x over previous
"""Trainium2 Bass kernel for ChannelAttention, v2 (single-read + fp16 store).

    k      = einsum('bcit,i->bct', signals, alpha)          # [B, C, T]
    scores = einsum('bct,ts,bds->bcd', k, Wc, k)            # [B, C, C]
    att    = softmax(scores, axis=-1)
    out    = einsum('bci,bint->bcnt', att, signals)         # [B, C, N, T]

Sharding: data-parallel over batch B=16 across 8 cores (2 per core).

Per-core plan (vs the 96 MiB/core double-read baseline):
  - signals read from HBM ONCE (32 MiB fp32), streamed into a per-batch
    fp16 SBUF resident tile R[(h c)=128, (n' t)=32768], n = h*256 + n'.
  - Phase A (k) on the PE from R: per n'-block n0, a small matmul with
    stationary selector sel_n0[p=(h c), c'] = alpha[h*256+n0] * (c==c'),
    accumulating k[c',t] over 256 blocks in PSUM.  sel table (4 MiB fp16)
    is built on-device from alpha via a PSUM broadcast matmul + 256 DVE
    tensor_scalar ops (interleaved with the stream).
  - Phase B: kT via PE transpose, scores = (k Wc) k^T, row softmax,
    att -> fp16, attT packed block-diagonally into [128,128] (both
    n-halves at once).
  - Phase C: out[(h c), :] = attT2.T @ R in 512-col matmuls; PSUM ->
    fp16 staging (ACT/DVE copies) -> HBM as fp16 (host upcasts).
  HBM traffic/core: 32 MiB in + 16 MiB out = 48 MiB (~134 us floor at
  358 GB/s) vs 96 MiB before.

Emission order is chosen for per-engine FIFO cleanliness:
  b0 stream+A | b0 B | b1 stream (DMA+downcast only) | b0 C (ACT copies
  only) | b1 A matmuls | b1 B | b1 C (DVE+ACT copies).
"""

import numpy as np
from contextlib import ExitStack

import concourse.bass as bass
import concourse.bacc as bacc
import concourse.tile as tile
import concourse.mybir as mybir
from concourse.bass_utils import run_bass_kernel_spmd
from concourse.masks import make_identity

B, C, N, T = 16, 64, 512, 128
NCORES = 8
BPC = B // NCORES
P = 128
NH = N // 2                # 256 n'-values per half
NT2 = NH * T               # 32768 cols per resident tile
CK = 2048                  # stream chunk cols (16 n'-blocks)
NCHUNK = NT2 // CK         # 16 chunks per batch
BLK = CK // T              # 16 n'-blocks per chunk
FP32 = mybir.dt.float32
FP16 = mybir.dt.float16

_PROGRAM_CACHE = {}


def _build_program() -> bass.Bass:
    nc = bacc.Bacc(None)
    sig_h = nc.declare_dram_parameter("signals", [BPC, C, N, T], FP32, isOutput=False)
    wc_h = nc.declare_dram_parameter("Wc", [T, T], FP32, isOutput=False)
    al_h = nc.declare_dram_parameter("alpha", [N], FP32, isOutput=False)
    out_h = nc.declare_dram_parameter("out", [BPC, C, N, T], FP16, isOutput=True)

    # per-batch, per-half [c, (n' t)] views; n = h*256 + n'.  SBUF rows
    # (h*64 + c) are fed by two DMAs per chunk, one per half.
    sig_hc = [
        sig_h.ap()[b].rearrange("c (h np) t -> h c (np t)", h=2)
        for b in range(BPC)
    ]
    out_hc = [
        out_h.ap()[b].rearrange("c (h np) t -> h c (np t)", h=2)
        for b in range(BPC)
    ]

    with ExitStack() as ctx:
        tc = ctx.enter_context(tile.TileContext(nc))
        singles = ctx.enter_context(tc.tile_pool(name="singles", bufs=1))
        rpool = ctx.enter_context(tc.tile_pool(name="rpool", bufs=2))
        opool = ctx.enter_context(tc.tile_pool(name="opool", bufs=2))
        small = ctx.enter_context(tc.tile_pool(name="small", bufs=2))
        pa = ctx.enter_context(tc.tile_pool(name="pa", bufs=1, space="PSUM"))
        pk = ctx.enter_context(tc.tile_pool(name="pk", bufs=1, space="PSUM"))
        pb = ctx.enter_context(tc.tile_pool(name="pb", bufs=2, space="PSUM"))
        po = ctx.enter_context(tc.tile_pool(name="po", bufs=4, space="PSUM"))

        # ---- constants
        wc_sb = singles.tile([T, T], FP32)
        nc.sync.dma_start(out=wc_sb, in_=wc_h.ap())
        al_sb = singles.tile([1, N], FP32)
        nc.sync.dma_start(out=al_sb, in_=al_h.ap().rearrange("(o n) -> o n", o=1))

        id64f = singles.tile([64, 64], FP32)
        make_identity(nc, id64f)
        id64h = singles.tile([64, 64], FP16)
        make_identity(nc, id64h)

        # D[p, c'] = (p % 64 == c'), two stacked I64
        d_sb = singles.tile([P, 64], FP16)
        nc.vector.tensor_copy(d_sb[0:64, :], id64h)
        nc.vector.tensor_copy(d_sb[64:128, :], id64h)

        # a2[p, n0] = alpha[(p//64)*256 + n0] via two K=1 broadcast matmuls
        mask0 = singles.tile([1, P], FP32)
        mask1 = singles.tile([1, P], FP32)
        nc.vector.memset(mask0, 0.0)
        nc.vector.memset(mask0[0:1, 0:64], 1.0)
        nc.vector.memset(mask1, 0.0)
        nc.vector.memset(mask1[0:1, 64:128], 1.0)
        a2_ps = pa.tile([P, NH], FP32, tag="pa")
        nc.tensor.matmul(a2_ps, lhsT=mask0, rhs=al_sb[:, 0:NH], start=True, stop=False)
        nc.tensor.matmul(a2_ps, lhsT=mask1, rhs=al_sb[:, NH:N], start=False, stop=True)
        a2_sb = singles.tile([P, NH], FP32)
        nc.vector.tensor_copy(a2_sb, a2_ps)

        # selector table: sel[p, n0*64 + c'] = a2[p, n0] * D[p, c']
        sel_sb = singles.tile([P, NH * 64], FP16)

        resid = []   # per-batch resident tiles

        CKB = 8192               # big-chunk cols (32 KiB/row descriptors)
        NBCH = NT2 // CKB        # 4 chunks per batch

        def stream_batch(b, with_sel, with_mms):
            R = rpool.tile([P, NT2], FP16, tag="R", name=f"R{b}")
            resid.append(R)
            kp = pk.tile([64, T], FP32, tag="k", name=f"k{b}") if with_mms else None
            for j in range(NBCH):
                if with_sel:
                    # sel[p, n0*64+c'] = a2[p, n0] * D[p, c'], quarter at a time
                    nq = NH // NBCH
                    q0 = j * nq
                    sv = sel_sb[:, q0 * 64:(q0 + nq) * 64].rearrange(
                        "p (n c) -> p n c", n=nq
                    )
                    nc.vector.tensor_tensor(
                        sv,
                        a2_sb[:, q0:q0 + nq].unsqueeze(2).broadcast_to([P, nq, 64]),
                        d_sb.unsqueeze(1).broadcast_to([P, nq, 64]),
                        mybir.AluOpType.mult,
                    )
                # casting SWDGE DMA: HBM fp32 -> SBUF fp16, no staging
                nc.gpsimd.dma_start(
                    out=R[0:64, j * CKB:(j + 1) * CKB],
                    in_=sig_hc[b][0, :, j * CKB:(j + 1) * CKB],
                )
                nc.gpsimd.dma_start(
                    out=R[64:128, j * CKB:(j + 1) * CKB],
                    in_=sig_hc[b][1, :, j * CKB:(j + 1) * CKB],
                )
                if with_mms:
                    phase_a_chunk(b, R, kp, j)
            return R, kp

        def phase_a_chunk(b, R, kp, j):
            nblk = CKB // T
            for u in range(nblk):
                n0 = j * nblk + u
                nc.tensor.matmul(
                    kp,
                    lhsT=sel_sb[:, n0 * 64:(n0 + 1) * 64],
                    rhs=R[:, n0 * T:(n0 + 1) * T],
                    start=(n0 == 0),
                    stop=(n0 == NH - 1),
                )

        def phase_b(b, kp):
            k_sb = small.tile([64, T], FP32, tag="ksb", name=f"ksb{b}")
            nc.vector.tensor_copy(k_sb, kp)
            ktp = pb.tile([T, 64], FP32, tag="pb", name=f"ktp{b}")
            nc.tensor.transpose(ktp, k_sb, id64f)
            kt_sb = small.tile([T, 64], FP32, tag="ktsb", name=f"ktsb{b}")
            nc.vector.tensor_copy(kt_sb, ktp)
            kwtp = pb.tile([T, 64], FP32, tag="pb", name=f"kwtp{b}")
            nc.tensor.matmul(kwtp, lhsT=wc_sb, rhs=kt_sb, start=True, stop=True)
            kwt_sb = small.tile([T, 64], FP32, tag="kwtsb", name=f"kwtsb{b}")
            nc.vector.tensor_copy(kwt_sb, kwtp)
            scp = pb.tile([64, 64], FP32, tag="pb", name=f"scp{b}")
            nc.tensor.matmul(scp, lhsT=kwt_sb, rhs=kt_sb, start=True, stop=True)

            mx = small.tile([64, 1], FP32, tag="mx", name=f"mx{b}")
            nmx = small.tile([64, 1], FP32, tag="nmx", name=f"nmx{b}")
            ssum = small.tile([64, 1], FP32, tag="ssum", name=f"ssum{b}")
            rsum = small.tile([64, 1], FP32, tag="rsum", name=f"rsum{b}")
            att_f = small.tile([64, 64], FP32, tag="attf", name=f"attf{b}")
            att_h_f32 = small.tile([64, 64], FP32, tag="atth", name=f"atth{b}")
            nc.vector.reduce_max(out=mx, in_=scp, axis=mybir.AxisListType.X)
            nc.vector.tensor_scalar_mul(nmx, mx, -1.0)
            nc.scalar.activation(
                att_f, scp, mybir.ActivationFunctionType.Exp,
                bias=nmx, scale=1.0, accum_out=ssum,
            )
            nc.vector.reciprocal(rsum, ssum)
            nc.scalar.mul(att_h_f32, att_f, rsum)
            atp = pb.tile([64, 64], FP32, tag="pb", name=f"atp{b}")
            nc.tensor.transpose(atp, att_h_f32, id64f)
            at2 = small.tile([P, P], FP16, tag="at2", name=f"at2{b}")
            nc.vector.memset(at2, 0.0)
            nc.vector.tensor_copy(at2[0:64, 0:64], atp)
            nc.vector.tensor_copy(at2[64:128, 64:128], atp)
            return at2

        GRP = 4096           # out staging cols (fp16)
        QPG = GRP // 512     # 512-col matmuls per group

        def phase_c_group(b, R, at2, g, copy_engines, h1_queue):
            stg = opool.tile([P, GRP], FP16, tag="o", name=f"o{b}_{g}")
            for q in range(QPG):
                ps = po.tile([P, 512], FP32, tag="po", name=f"po{b}_{g}_{q}")
                nc.tensor.matmul(
                    ps, lhsT=at2,
                    rhs=R[:, (g * QPG + q) * 512:(g * QPG + q + 1) * 512],
                    start=True, stop=True,
                )
                eng = copy_engines[q % len(copy_engines)]
                if eng == "v":
                    nc.vector.tensor_copy(stg[:, q * 512:(q + 1) * 512], ps)
                else:
                    nc.scalar.copy(stg[:, q * 512:(q + 1) * 512], ps)
            nc.sync.dma_start(
                out=out_hc[b][0, :, g * GRP:(g + 1) * GRP], in_=stg[0:64, :]
            )
            h1_queue.dma_start(
                out=out_hc[b][1, :, g * GRP:(g + 1) * GRP], in_=stg[64:128, :]
            )

        def phase_c(b, R, at2, copy_engines, h1_queue):
            for g in range(NT2 // GRP):
                phase_c_group(b, R, at2, g, copy_engines, h1_queue)

        # ---- emission schedule.  b1's stream chunks, b0's phase-C groups
        # and b1's phase-A matmuls are interleaved so the PE/DVE/ACT FIFOs
        # never serialize the b1 tail behind the whole of b0's phase C.
        R0, kp0 = stream_batch(0, with_sel=True, with_mms=True)
        at2_0 = phase_b(0, kp0)
        R1 = rpool.tile([P, NT2], FP16, tag="R", name="R1")
        kp1 = pk.tile([64, T], FP32, tag="k", name="k1")
        GPB = (NT2 // GRP) // NBCH      # b0 phase-C groups per b1 chunk
        for j in range(NBCH):
            nc.gpsimd.dma_start(
                out=R1[0:64, j * CKB:(j + 1) * CKB],
                in_=sig_hc[1][0, :, j * CKB:(j + 1) * CKB],
            )
            nc.gpsimd.dma_start(
                out=R1[64:128, j * CKB:(j + 1) * CKB],
                in_=sig_hc[1][1, :, j * CKB:(j + 1) * CKB],
            )
            # phase-A matmuls for the PREVIOUS chunk: its data already
            # landed, so these never head-block the C groups behind them.
            if j > 0:
                phase_a_chunk(1, R1, kp1, j - 1)
            for g in range(j * GPB, (j + 1) * GPB):
                phase_c_group(0, R0, at2_0, g,
                              copy_engines=["v", "v", "v", "s"],
                              h1_queue=nc.scalar)
        phase_a_chunk(1, R1, kp1, NBCH - 1)
        at2_1 = phase_b(1, kp1)
        phase_c(1, R1, at2_1, copy_engines=["v", "s"], h1_queue=nc.gpsimd)

    nc.compile()
    return nc


def _get_program() -> bass.Bass:
    if "nc" not in _PROGRAM_CACHE:
        _PROGRAM_CACHE["nc"] = _build_program()
    return _PROGRAM_CACHE["nc"]


def kernel(signals, Wc, alpha, **run_kwargs):
    signals = np.ascontiguousarray(np.asarray(signals, dtype=np.float32))
    Wc = np.ascontiguousarray(np.asarray(Wc, dtype=np.float32))
    alpha = np.ascontiguousarray(np.asarray(alpha, dtype=np.float32))
    assert signals.shape == (B, C, N, T)

    nc = _get_program()
    core_ids = list(range(NCORES))
    in_maps = [
        {
            "signals": signals[j * BPC:(j + 1) * BPC],
            "Wc": Wc,
            "alpha": alpha,
        }
        for j in range(NCORES)
    ]
    res = run_bass_kernel_spmd(nc, in_maps, core_ids, **run_kwargs)
    out = np.empty((B, C, N, T), dtype=np.float32)
    for j in range(NCORES):
        out[j * BPC:(j + 1) * BPC] = np.asarray(res.results[j]["out"], dtype=np.float32)
    if run_kwargs:
        kernel.last_results = res
    return out
